# revision 2
# baseline (speedup 1.0000x reference)
"""Trainium2 Bass kernel for a GPT-style decoder block (S=2048, E=2048, H=16, D=128).

Sharding: sequence-parallel across 8 NeuronCores. Core c owns row-blocks
(2c, 2c+1) of 128 rows each. Everything (LN1, QKV, attention queries,
attn-proj, LN2, FFN) is computed row-parallel on the owned 256 rows; the
only collective is a single AllGather of the (bf16) K^T and V projections
so every core can attend over the full key space. Causality is enforced
with per-core mask inputs so the program is SPMD-uniform across cores.

All matmuls run in bf16 with fp32 PSUM accumulation; the residual stream
stays fp32 in SBUF end-to-end. LayerNorm scale vectors are folded into the
following weight matrices on the host (exact for ln_w == 1).
"""

import numpy as np
import ml_dtypes

import concourse.bass as bass
import concourse.mybir as mybir
import concourse.tile as tile
from concourse import bacc
from concourse.bass_utils import run_bass_kernel_spmd

P = 128
S, E, H, D = 2048, 2048, 16, 128
FH = 4 * E          # 8192 ffn hidden
NCORES = 8
NBLK = 16           # S / P row blocks
EC = E // P         # 16 contraction chunks over E
FC2 = FH // P       # 64 f-chunks for the fc matmul
BF = mybir.dt.bfloat16
F32 = mybir.dt.float32
EPS = 1e-5
SCALE = 1.0 / np.sqrt(D)
AF = mybir.ActivationFunctionType
ALU = mybir.AluOpType

KT_ELEMS = H * D * 256          # per-rank K^T contribution elements
V_ELEMS = 256 * E               # per-rank V contribution elements
CONTRIB = KT_ELEMS + V_ELEMS


def _layer_norm_to_bf16(nc, small, x_sb, h_out, eps_tile, tag):
    """h_out[P, E](bf16) = (x - mean(x)) * rsqrt(var(x) + eps), rowwise over E."""
    stats = small.tile([P, 4, 6], F32, name=f"stats_{tag}", tag="stats")
    for g in range(4):
        nc.vector.bn_stats(out=stats[:, g, :], in_=x_sb[:, g * 512:(g + 1) * 512])
    mv = small.tile([P, 2], F32, name=f"mv_{tag}", tag="mv")
    nc.vector.bn_aggr(out=mv[:], in_=stats[:])
    std = small.tile([P, 1], F32, name=f"std_{tag}", tag="std")
    nc.scalar.activation(out=std[:], in_=mv[:, 1:2], func=AF.Sqrt,
                         bias=eps_tile[:], scale=1.0)
    rstd = small.tile([P, 1], F32, name=f"rstd_{tag}", tag="rstd")
    nc.vector.reciprocal(out=rstd[:], in_=std[:])
    nc.vector.tensor_scalar(
        out=h_out[:], in0=x_sb[:], scalar1=mv[:, 0:1], scalar2=rstd[:],
        op0=ALU.subtract, op1=ALU.mult)


def build_program():
    nc = bacc.Bacc()

    # ---- external I/O (per-core views; host pre-tiles the weights) ----
    x_own = nc.dram_tensor("x_own", [2, P, E], F32, kind="ExternalInput")
    # wqk[fc][ei, eo, f] = w_attn[fc*128+f, eo*128+ei] (q rows then k rows)
    wqk = nc.dram_tensor("wqk", [32, P, EC, P], BF, kind="ExternalInput")
    # wv[dc][e][ei, f] = w_attn[4096 + dc*512 + f, e*128+ei]
    wv = nc.dram_tensor("wv", [4, EC, P, 512], BF, kind="ExternalInput")
    # wpa[n][hc][i, f] = w_proj_attn[n*512+f, hc*128+i]
    wpa = nc.dram_tensor("wpa", [4, EC, P, 512], BF, kind="ExternalInput")
    # wfc[f2][ei, eo, f] = w_fc[f2*128+f, eo*128+ei]
    wfc = nc.dram_tensor("wfc", [FC2, P, EC, P], BF, kind="ExternalInput")
    # wpf[n][f2][fi, f] = w_proj_ffn[n*512+f, f2*128+fi]
    wpf = nc.dram_tensor("wpf", [4, FC2, P, 512], BF, kind="ExternalInput")
    ident_in = nc.dram_tensor("ident", [P, P], BF, kind="ExternalInput")
    # mask[j][t, tb, s]: 1 where key (tb*128+t) <= query (bq*128+s) for this
    # core's j-th owned block bq, else 0.
    mask_in = nc.dram_tensor("mask", [2, P, NBLK, P], BF, kind="ExternalInput")
    out_own = nc.dram_tensor("out_own", [2, P, E], F32, kind="ExternalOutput")

    with tile.TileContext(nc) as tc:
        _body(nc, tc, x_own, wqk, wv, wpa, wfc, wpf, ident_in, mask_in, out_own)
    nc.finalize()
    return nc


def _body(nc, tc, x_own, wqk, wv, wpa, wfc, wpf, ident_in, mask_in, out_own):
    with tc.tile_pool(name="resident", bufs=1) as res, \
         tc.tile_pool(name="small", bufs=4) as small, \
         tc.tile_pool(name="dram", bufs=1, space="DRAM") as dram:

        # ---------- constants / residents ----------
        eps_tile = small.tile([P, 1], F32, name="eps_tile", tag="eps")
        nc.vector.memset(eps_tile[:], EPS)
        ident = res.tile([P, P], BF, name="ident_sb")
        nc.sync.dma_start(ident[:], ident_in[:])
        mask_sb = []
        for j in range(2):
            m = res.tile([P, NBLK, P], BF, name=f"mask_sb{j}")
            nc.sync.dma_start(m[:], mask_in[j])
            mask_sb.append(m)

        x_sb = []
        for j in range(2):
            x = res.tile([P, E], F32, name=f"x_sb{j}")
            nc.sync.dma_start(x[:], x_own[j])
            x_sb.append(x)

        hT = res.tile([P, EC, 256], BF, name="hT_sb")
        qT = res.tile([P, H, 256], BF, name="qT_sb")
        attnT = res.tile([P, H, 256], BF, name="attnT_sb")
        h2T = res.tile([P, EC, 256], BF, name="h2T_sb")

        cc_in = dram.tile([CONTRIB], BF, name="cc_in")
        cc_out = dram.tile([NCORES, CONTRIB], BF, name="cc_out",
                           addr_space="Shared")
        cc_in_kt = cc_in[:KT_ELEMS].rearrange("(h d s) -> h d s", h=H, d=D)
        cc_in_v = cc_in[KT_ELEMS:].rearrange("(s e) -> s e", s=256)

        # ---------- P1+P2: LN1 -> h (bf16) -> hT ----------
        with tc.tile_pool(name="hbf", bufs=2) as hbf_pool, \
             tc.tile_pool(name="tps1", bufs=3, space="PSUM") as tps1:
            for j in range(2):
                h_bf = hbf_pool.tile([P, E], BF, name="h_bf", tag="h_bf")
                _layer_norm_to_bf16(nc, small, x_sb[j], h_bf, eps_tile, f"ln1_{j}")
                for e in range(EC):
                    tp = tps1.tile([P, P], BF, name="tp1", tag="tp1")
                    nc.tensor.transpose(tp[:], h_bf[:, e * P:(e + 1) * P], ident[:])
                    nc.vector.tensor_copy(hT[:, e, j * P:(j + 1) * P], tp[:])

        # ---------- P3: k, v (to AG input), then q ----------
        with tc.tile_pool(name="wqk_sb", bufs=3) as wqk_pool, \
             tc.tile_pool(name="wv_sb", bufs=6) as wv_pool, \
             tc.tile_pool(name="kvtmp", bufs=3) as kv_pool, \
             tc.tile_pool(name="qkps", bufs=3, space="PSUM") as qkps, \
             tc.tile_pool(name="vps", bufs=2, space="PSUM") as vps:

            def qk_chunk(fc):
                wt = wqk_pool.tile([P, EC, P], BF, name="wqk_t", tag="wqk_t")
                nc.sync.dma_start(wt[:], wqk[fc])
                ps = qkps.tile([P, 256], F32, name="qk_ps", tag="qk_ps")
                for e in range(EC):
                    nc.tensor.matmul(ps[:], wt[:, e, :], hT[:, e, :],
                                     start=(e == 0), stop=(e == EC - 1))
                return ps

            # k chunks first so the AllGather can start as early as possible
            for h in range(H):
                ps = qk_chunk(16 + h)
                kt_sb = kv_pool.tile([P, 256], BF, name="kt_sb", tag="kt_sb")
                nc.vector.tensor_copy(kt_sb[:], ps[:])
                nc.sync.dma_start(cc_in_kt[h], kt_sb[:])

            # v: out[s, d] with lhsT = hT chunk, rhs = wv tile
            for j in range(2):
                for dc in range(4):
                    ps = vps.tile([P, 512], F32, name="v_ps", tag="v_ps")
                    for e in range(EC):
                        wvt = wv_pool.tile([P, 512], BF, name="wv_t", tag="wv_t")
                        nc.sync.dma_start(wvt[:], wv[dc, e])
                        nc.tensor.matmul(ps[:], hT[:, e, j * P:(j + 1) * P], wvt[:],
                                         start=(e == 0), stop=(e == EC - 1))
                    v_sb = kv_pool.tile([P, 512], BF, name="v_sb", tag="v_sb")
                    nc.vector.tensor_copy(v_sb[:], ps[:])
                    nc.sync.dma_start(
                        cc_in_v[j * P:(j + 1) * P, dc * 512:(dc + 1) * 512], v_sb[:])

            nc.gpsimd.collective_compute(
                "AllGather", ALU.bypass,
                replica_groups=[list(range(NCORES))],
                ins=[cc_in.opt()], outs=[cc_out.opt()])

            # q chunks overlap the AllGather
            for h in range(H):
                ps = qk_chunk(h)
                nc.vector.tensor_copy(qT[:, h, :], ps[:])

        ccout_kt = cc_out[:, :KT_ELEMS].rearrange(
            "r (h d b s) -> r h d b s", h=H, d=D, b=2)
        ccout_v = cc_out[:, KT_ELEMS:].rearrange(
            "r (b t e) -> r b t e", b=2, t=P)

        # ---------- P5: attention ----------
        with tc.tile_pool(name="vall", bufs=1) as vall_pool, \
             tc.tile_pool(name="kth", bufs=3) as kt_pool, \
             tc.tile_pool(name="expt", bufs=3) as exp_pool, \
             tc.tile_pool(name="attn_small", bufs=4) as asmall, \
             tc.tile_pool(name="scps", bufs=3, space="PSUM") as scps, \
             tc.tile_pool(name="atps", bufs=2, space="PSUM") as atps, \
             tc.tile_pool(name="trps", bufs=2, space="PSUM") as trps:

            # V for all heads, resident: V_all[tb][t, h, 0:128]=V, [:, :, 128]=1
            v_all = []
            for tb in range(NBLK):
                r, b = tb // 2, tb % 2
                vt = vall_pool.tile([P, H, D + 1], BF, name=f"v_all_{tb}")
                nc.sync.dma_start(
                    vt[:, :, :D], ccout_v[r, b].rearrange("t (h d) -> t h d", h=H))
                nc.vector.memset(vt[:, :, D:D + 1], 1.0)
                v_all.append(vt)

            for h in range(H):
                kt_h = kt_pool.tile([P, NBLK, P], BF, name="kt_h", tag="kt_h")
                for r in range(NCORES):
                    nc.sync.dma_start(kt_h[:, 2 * r:2 * r + 2, :], ccout_kt[r, h])
                for j in range(2):
                    expT = exp_pool.tile([P, NBLK, P], BF, name="expT", tag="expT")
                    for tb in range(NBLK):
                        ps_sc = scps.tile([P, P], F32, name="sc_ps", tag="sc_ps")
                        nc.tensor.matmul(ps_sc[:], kt_h[:, tb, :],
                                         qT[:, h, j * P:(j + 1) * P],
                                         start=True, stop=True)
                        nc.scalar.activation(out=expT[:, tb, :], in_=ps_sc[:],
                                             func=AF.Exp, scale=float(SCALE))
                    # zero non-causal probabilities (one fused multiply)
                    nc.vector.tensor_mul(expT[:], expT[:], mask_sb[j][:])
                    ps_at = atps.tile([P, D + 1], F32, name="at_ps", tag="at_ps")
                    for tb in range(NBLK):
                        nc.tensor.matmul(ps_at[:], expT[:, tb, :],
                                         v_all[tb][:, h, :],
                                         start=(tb == 0), stop=(tb == NBLK - 1))
                    recip = asmall.tile([P, 1], F32, name="recip", tag="recip")
                    nc.vector.reciprocal(recip[:], ps_at[:, D:D + 1])
                    a_sb = asmall.tile([P, P], BF, name="a_sb", tag="a_sb")
                    nc.vector.tensor_scalar_mul(a_sb[:], ps_at[:, :D], recip[:])
                    tp = trps.tile([P, P], BF, name="tp_at", tag="tp_at")
                    nc.tensor.transpose(tp[:], a_sb[:], ident[:])
                    nc.vector.tensor_copy(attnT[:, h, j * P:(j + 1) * P], tp[:])

        # ---------- P6: attn out-proj + residual ----------
        with tc.tile_pool(name="wpa_sb", bufs=6) as wpa_pool, \
             tc.tile_pool(name="pps", bufs=4, space="PSUM") as pps_pool:
            for n in range(4):
                ps = [pps_pool.tile([P, 512], F32, name=f"p_ps{j}", tag="p_ps")
                      for j in range(2)]
                for hc in range(EC):
                    wt = wpa_pool.tile([P, 512], BF, name="wpa_t", tag="wpa_t")
                    nc.sync.dma_start(wt[:], wpa[n, hc])
                    for j in range(2):
                        nc.tensor.matmul(ps[j][:], attnT[:, hc, j * P:(j + 1) * P],
                                         wt[:], start=(hc == 0), stop=(hc == EC - 1))
                for j in range(2):
                    nc.vector.tensor_add(
                        out=x_sb[j][:, n * 512:(n + 1) * 512],
                        in0=x_sb[j][:, n * 512:(n + 1) * 512], in1=ps[j][:])

        # ---------- P7: LN2 -> h2T ----------
        with tc.tile_pool(name="hbf2", bufs=2) as hbf2_pool, \
             tc.tile_pool(name="tps2", bufs=3, space="PSUM") as tps2:
            for j in range(2):
                h_bf = hbf2_pool.tile([P, E], BF, name="h2_bf", tag="h2_bf")
                _layer_norm_to_bf16(nc, small, x_sb[j], h_bf, eps_tile, f"ln2_{j}")
                for e in range(EC):
                    tp = tps2.tile([P, P], BF, name="tp2", tag="tp2")
                    nc.tensor.transpose(tp[:], h_bf[:, e * P:(e + 1) * P], ident[:])
                    nc.vector.tensor_copy(h2T[:, e, j * P:(j + 1) * P], tp[:])

        # ---------- P8: fc + gelu -> gT ----------
        gT = res.tile([P, FC2, 256], BF, name="gT_sb")
        with tc.tile_pool(name="wfc_sb", bufs=3) as wfc_pool, \
             tc.tile_pool(name="fcps", bufs=4, space="PSUM") as fcps:
            for f2 in range(FC2):
                wt = wfc_pool.tile([P, EC, P], BF, name="wfc_t", tag="wfc_t")
                nc.sync.dma_start(wt[:], wfc[f2])
                ps = fcps.tile([P, 256], F32, name="fc_ps", tag="fc_ps")
                for e in range(EC):
                    nc.tensor.matmul(ps[:], wt[:, e, :], h2T[:, e, :],
                                     start=(e == 0), stop=(e == EC - 1))
                nc.scalar.activation(out=gT[:, f2, :], in_=ps[:],
                                     func=AF.Gelu_apprx_tanh)

        # ---------- P9: ffn out-proj + residual, P10: store ----------
        with tc.tile_pool(name="wpf_sb", bufs=8) as wpf_pool, \
             tc.tile_pool(name="pfps", bufs=4, space="PSUM") as pfps:
            for n in range(4):
                ps = [pfps.tile([P, 512], F32, name=f"pf_ps{j}", tag="pf_ps")
                      for j in range(2)]
                for f2 in range(FC2):
                    wt = wpf_pool.tile([P, 512], BF, name="wpf_t", tag="wpf_t")
                    nc.sync.dma_start(wt[:], wpf[n, f2])
                    for j in range(2):
                        nc.tensor.matmul(ps[j][:], gT[:, f2, j * P:(j + 1) * P],
                                         wt[:], start=(f2 == 0),
                                         stop=(f2 == FC2 - 1))
                for j in range(2):
                    nc.vector.tensor_add(
                        out=x_sb[j][:, n * 512:(n + 1) * 512],
                        in0=x_sb[j][:, n * 512:(n + 1) * 512], in1=ps[j][:])
            for j in range(2):
                nc.sync.dma_start(out_own[j], x_sb[j][:])


# ------------------------------------------------------------------
# host side
# ------------------------------------------------------------------
_BF = ml_dtypes.bfloat16


def _prep_shared(ln1_w, ln2_w, w_attn, w_proj_attn, w_fc, w_proj_ffn):
    w_attn = (w_attn * ln1_w[None, :]).astype(np.float32)
    w_fc = (w_fc * ln2_w[None, :]).astype(np.float32)
    wqk = np.ascontiguousarray(
        w_attn[:2 * E].reshape(32, P, EC, P).transpose(0, 3, 2, 1)).astype(_BF)
    wv = np.ascontiguousarray(
        w_attn[2 * E:].reshape(4, 512, EC, P).transpose(0, 2, 3, 1)).astype(_BF)
    wpa = np.ascontiguousarray(
        w_proj_attn.reshape(4, 512, EC, P).transpose(0, 2, 3, 1)).astype(_BF)
    wfc_t = np.ascontiguousarray(
        w_fc.reshape(FC2, P, EC, P).transpose(0, 3, 2, 1)).astype(_BF)
    wpf = np.ascontiguousarray(
        w_proj_ffn.reshape(4, 512, FC2, P).transpose(0, 2, 3, 1)).astype(_BF)
    ident = np.eye(P, dtype=np.float32).astype(_BF)
    return wqk, wv, wpa, wfc_t, wpf, ident


def _core_masks(c):
    """mask[j][t, tb, s] = 1 iff key tb*128+t <= query (2c+j)*128+s."""
    t = np.arange(P)
    s = np.arange(P)
    masks = np.zeros((2, P, NBLK, P), np.float32)
    for j in range(2):
        bq = 2 * c + j
        for tb in range(NBLK):
            if tb < bq:
                masks[j, :, tb, :] = 1.0
            elif tb == bq:
                masks[j, :, tb, :] = (t[:, None] <= s[None, :])
    return masks.astype(_BF)


_CACHE = {}


def _get_program():
    if "nc" not in _CACHE:
        _CACHE["nc"] = build_program()
    return _CACHE["nc"]


def make_in_maps(x, ln1_w, ln2_w, w_attn, w_proj_attn, w_fc, w_proj_ffn):
    wqk, wv, wpa, wfc_t, wpf, ident = _prep_shared(
        np.asarray(ln1_w, np.float32), np.asarray(ln2_w, np.float32),
        np.asarray(w_attn, np.float32), np.asarray(w_proj_attn, np.float32),
        np.asarray(w_fc, np.float32), np.asarray(w_proj_ffn, np.float32))
    xb = np.ascontiguousarray(np.asarray(x, np.float32).reshape(NBLK, P, E))
    in_maps = []
    for c in range(NCORES):
        in_maps.append({
            "x_own": np.ascontiguousarray(xb[2 * c:2 * c + 2]),
            "wqk": wqk, "wv": wv, "wpa": wpa, "wfc": wfc_t, "wpf": wpf,
            "ident": ident, "mask": _core_masks(c),
        })
    return in_maps


def kernel(x, ln1_w, ln2_w, w_attn, w_proj_attn, w_fc, w_proj_ffn):
    nc = _get_program()
    in_maps = make_in_maps(x, ln1_w, ln2_w, w_attn, w_proj_attn, w_fc,
                           w_proj_ffn)
    res = run_bass_kernel_spmd(nc, in_maps, core_ids=list(range(NCORES)))
    out = np.empty((S, E), np.float32)
    for c in range(NCORES):
        blk = res.results[c]["out_own"]
        out[2 * c * P:(2 * c + 1) * P] = blk[0]
        out[(2 * c + 1) * P:(2 * c + 2) * P] = blk[1]
    return out


if __name__ == "__main__":
    rng = np.random.default_rng(0)
    ins = {
        "x": rng.standard_normal((S, E), dtype=np.float32),
        "ln1_w": np.ones(E, np.float32),
        "ln2_w": np.ones(E, np.float32),
        "w_attn": (rng.standard_normal((3 * E, E), dtype=np.float32) * 0.02),
        "w_proj_attn": (rng.standard_normal((E, E), dtype=np.float32) * 0.02),
        "w_fc": (rng.standard_normal((FH, E), dtype=np.float32) * 0.02),
        "w_proj_ffn": (rng.standard_normal((E, FH), dtype=np.float32) * 0.02),
    }
    out = kernel(**ins)
    print("ran:", out.shape, out.dtype, np.abs(out).max())


# revision 5
# speedup vs baseline: 1.1820x; 1.1820x over previous
"""Trainium2 Bass kernel for a GPT-style decoder block (S=2048, E=2048, H=16, D=128).

Sharding: sequence-parallel across 8 NeuronCores. Core c owns row-blocks
(2c, 2c+1) of 128 rows each. Everything (LN1, QKV, attention queries,
attn-proj, LN2, FFN) is computed row-parallel on the owned 256 rows; the
only collective is a single AllGather of the (bf16) K^T and V projections
so every core can attend over the full key space. Causality is enforced
with per-core mask inputs so the program is SPMD-uniform across cores.

All matmuls run in bf16 with fp32 PSUM accumulation; the residual stream
stays fp32 in SBUF end-to-end. LayerNorm scale vectors are folded into the
following weight matrices on the host (exact for ln_w == 1).
"""

import numpy as np
import ml_dtypes

import concourse.bass as bass
import concourse.mybir as mybir
import concourse.tile as tile
from concourse import bacc
from concourse.bass_utils import run_bass_kernel_spmd

P = 128
S, E, H, D = 2048, 2048, 16, 128
FH = 4 * E          # 8192 ffn hidden
NCORES = 8
NBLK = 16           # S / P row blocks
EC = E // P         # 16 contraction chunks over E
FC2 = FH // P       # 64 f-chunks for the fc matmul
BF = mybir.dt.bfloat16
F32 = mybir.dt.float32
EPS = 1e-5
SCALE = 1.0 / np.sqrt(D)
AF = mybir.ActivationFunctionType
ALU = mybir.AluOpType

KT_ELEMS = H * D * 256          # per-rank K^T contribution elements
V_ELEMS = 256 * E               # per-rank V contribution elements
CONTRIB = KT_ELEMS + V_ELEMS


def _layer_norm_to_bf16(nc, small, x_sb, h_out, eps_tile, tag):
    """h_out[P, E](bf16) = (x - mean(x)) * rsqrt(var(x) + eps), rowwise over E."""
    stats = small.tile([P, 4, 6], F32, name=f"stats_{tag}", tag="stats")
    for g in range(4):
        nc.vector.bn_stats(out=stats[:, g, :], in_=x_sb[:, g * 512:(g + 1) * 512])
    mv = small.tile([P, 2], F32, name=f"mv_{tag}", tag="mv")
    nc.vector.bn_aggr(out=mv[:], in_=stats[:])
    std = small.tile([P, 1], F32, name=f"std_{tag}", tag="std")
    nc.scalar.activation(out=std[:], in_=mv[:, 1:2], func=AF.Sqrt,
                         bias=eps_tile[:], scale=1.0)
    rstd = small.tile([P, 1], F32, name=f"rstd_{tag}", tag="rstd")
    nc.vector.reciprocal(out=rstd[:], in_=std[:])
    nc.vector.tensor_scalar(
        out=h_out[:], in0=x_sb[:], scalar1=mv[:, 0:1], scalar2=rstd[:],
        op0=ALU.subtract, op1=ALU.mult)


def build_program():
    nc = bacc.Bacc()

    # ---- external I/O (per-core views; host pre-tiles the weights) ----
    x_own = nc.dram_tensor("x_own", [2, P, E], F32, kind="ExternalInput")
    # wqk[fc][ei, eo, f] = w_attn[fc*128+f, eo*128+ei] (q rows then k rows)
    wqk = nc.dram_tensor("wqk", [32, P, EC, P], BF, kind="ExternalInput")
    # wv[half][e][ei, dci, f] = w_attn[4096 + (2*half+dci)*512 + f, e*128+ei]
    wv = nc.dram_tensor("wv", [2, EC, P, 2, 512], BF, kind="ExternalInput")
    # wpa[hc][i, n, f] = w_proj_attn[n*512+f, hc*128+i]
    wpa = nc.dram_tensor("wpa", [EC, P, 4, 512], BF, kind="ExternalInput")
    # wfc[f2][ei, eo, f] = w_fc[f2*128+f, eo*128+ei]
    wfc = nc.dram_tensor("wfc", [FC2, P, EC, P], BF, kind="ExternalInput")
    # wpf[f2][fi, n, f] = w_proj_ffn[n*512+f, f2*128+fi]
    wpf = nc.dram_tensor("wpf", [FC2, P, 4, 512], BF, kind="ExternalInput")
    ident_in = nc.dram_tensor("ident", [P, P], BF, kind="ExternalInput")
    # mask[t, tb, s2]: s2 = j*128+s; 1 where key tb*128+t <= query (2c+j)*128+s
    mask_in = nc.dram_tensor("mask", [P, NBLK, 256], BF, kind="ExternalInput")
    out_own = nc.dram_tensor("out_own", [2, P, E], F32, kind="ExternalOutput")

    with tile.TileContext(nc) as tc:
        _body(nc, tc, x_own, wqk, wv, wpa, wfc, wpf, ident_in, mask_in, out_own)
    nc.finalize()
    return nc


def _body(nc, tc, x_own, wqk, wv, wpa, wfc, wpf, ident_in, mask_in, out_own):
    with tc.tile_pool(name="resident", bufs=1) as res, \
         tc.tile_pool(name="small", bufs=4) as small, \
         tc.tile_pool(name="dram", bufs=1, space="DRAM") as dram:

        # ---------- constants / residents ----------
        eps_tile = small.tile([P, 1], F32, name="eps_tile", tag="eps")
        nc.vector.memset(eps_tile[:], EPS)
        ident = res.tile([P, P], BF, name="ident_sb")
        nc.sync.dma_start(ident[:], ident_in[:])
        mask_sb = res.tile([P, NBLK, 256], BF, name="mask_sb")
        nc.sync.dma_start(mask_sb[:], mask_in[:])

        x_sb = []
        for j in range(2):
            x = res.tile([P, E], F32, name=f"x_sb{j}")
            nc.sync.dma_start(x[:], x_own[j])
            x_sb.append(x)

        hT = res.tile([P, EC, 256], BF, name="hT_sb")
        qT = res.tile([P, H, 256], BF, name="qT_sb")
        attnT = res.tile([P, H, 256], BF, name="attnT_sb")
        h2T = res.tile([P, EC, 256], BF, name="h2T_sb")

        cc_in = dram.tile([CONTRIB], BF, name="cc_in")
        cc_out = dram.tile([NCORES, CONTRIB], BF, name="cc_out",
                           addr_space="Shared")
        cc_in_kt = cc_in[:KT_ELEMS].rearrange("(h d s) -> h d s", h=H, d=D)
        cc_in_v = cc_in[KT_ELEMS:].rearrange("(s e) -> s e", s=256)

        # ---------- P1+P2: LN1 -> h (bf16) -> hT ----------
        with tc.tile_pool(name="hbf", bufs=2) as hbf_pool, \
             tc.tile_pool(name="tps1", bufs=3, space="PSUM") as tps1:
            for j in range(2):
                h_bf = hbf_pool.tile([P, E], BF, name="h_bf", tag="h_bf")
                _layer_norm_to_bf16(nc, small, x_sb[j], h_bf, eps_tile, f"ln1_{j}")
                for e in range(EC):
                    tp = tps1.tile([P, P], BF, name="tp1", tag="tp1")
                    nc.tensor.transpose(tp[:], h_bf[:, e * P:(e + 1) * P], ident[:])
                    nc.vector.tensor_copy(hT[:, e, j * P:(j + 1) * P], tp[:])

        # ---------- P3: k, v (to AG input), then q ----------
        with tc.tile_pool(name="wqk_sb", bufs=3) as wqk_pool, \
             tc.tile_pool(name="wv_sb", bufs=4) as wv_pool, \
             tc.tile_pool(name="kvtmp", bufs=3) as kv_pool, \
             tc.tile_pool(name="qkps", bufs=2, space="PSUM") as qkps, \
             tc.tile_pool(name="vps", bufs=1, space="PSUM") as vps:

            def qk_chunk(fc):
                wt = wqk_pool.tile([P, EC, P], BF, name="wqk_t", tag="wqk_t")
                nc.sync.dma_start(wt[:], wqk[fc])
                ps = qkps.tile([P, 256], F32, name="qk_ps", tag="qk_ps")
                for e in range(EC):
                    nc.tensor.matmul(ps[:], wt[:, e, :], hT[:, e, :],
                                     start=(e == 0), stop=(e == EC - 1))
                return ps

            # k chunks first so the AllGather can start as early as possible
            for h in range(H):
                ps = qk_chunk(16 + h)
                kt_sb = kv_pool.tile([P, 256], BF, name="kt_sb", tag="kt_sb")
                nc.vector.tensor_copy(kt_sb[:], ps[:])
                nc.sync.dma_start(cc_in_kt[h], kt_sb[:])

            # v: out[s, d] with lhsT = hT chunk, rhs = wv tile
            for half in range(2):
                ps_v = [[vps.tile([P, 512], F32, name=f"v_ps{j}_{dci}",
                                  tag=f"v_ps{j}_{dci}") for dci in range(2)]
                        for j in range(2)]
                for e in range(EC):
                    wvt = wv_pool.tile([P, 2, 512], BF, name="wv_t", tag="wv_t")
                    nc.sync.dma_start(wvt[:], wv[half, e])
                    for j in range(2):
                        for dci in range(2):
                            nc.tensor.matmul(
                                ps_v[j][dci][:], hT[:, e, j * P:(j + 1) * P],
                                wvt[:, dci, :],
                                start=(e == 0), stop=(e == EC - 1))
                for j in range(2):
                    for dci in range(2):
                        dc = 2 * half + dci
                        v_sb = kv_pool.tile([P, 512], BF, name="v_sb", tag="v_sb")
                        nc.vector.tensor_copy(v_sb[:], ps_v[j][dci][:])
                        nc.sync.dma_start(
                            cc_in_v[j * P:(j + 1) * P, dc * 512:(dc + 1) * 512],
                            v_sb[:])

            nc.gpsimd.collective_compute(
                "AllGather", ALU.bypass,
                replica_groups=[list(range(NCORES))],
                ins=[cc_in.opt()], outs=[cc_out.opt()])

            # q chunks overlap the AllGather
            for h in range(H):
                ps = qk_chunk(h)
                nc.vector.tensor_copy(qT[:, h, :], ps[:])

        ccout_kt = cc_out[:, :KT_ELEMS].rearrange(
            "r (h d b s) -> r h d b s", h=H, d=D, b=2)
        ccout_v = cc_out[:, KT_ELEMS:].rearrange(
            "r (b t e) -> r b t e", b=2, t=P)

        # ---------- P5: attention ----------
        with tc.tile_pool(name="vall", bufs=1) as vall_pool, \
             tc.tile_pool(name="kth", bufs=3) as kt_pool, \
             tc.tile_pool(name="expt", bufs=2) as exp_pool, \
             tc.tile_pool(name="attn_small", bufs=4) as asmall, \
             tc.tile_pool(name="scps", bufs=3, space="PSUM") as scps, \
             tc.tile_pool(name="atps", bufs=2, space="PSUM") as atps, \
             tc.tile_pool(name="trps", bufs=2, space="PSUM") as trps:

            # V for all heads, resident: V_all[tb][t, h, 0:128]=V, [:, :, 128]=1
            v_all = []
            for tb in range(NBLK):
                r, b = tb // 2, tb % 2
                vt = vall_pool.tile([P, H, D + 1], BF, name=f"v_all_{tb}")
                nc.sync.dma_start(
                    vt[:, :, :D], ccout_v[r, b].rearrange("t (h d) -> t h d", h=H))
                nc.vector.memset(vt[:, :, D:D + 1], 1.0)
                v_all.append(vt)

            for h in range(H):
                kt_h = kt_pool.tile([P, NBLK, P], BF, name="kt_h", tag="kt_h")
                nc.sync.dma_start(
                    kt_h[:], ccout_kt[:, h].rearrange("r d b s -> d r b s"))
                # scores + exp for both query blocks at once (free dim 256)
                expT = exp_pool.tile([P, NBLK, 256], BF, name="expT", tag="expT")
                for tb in range(NBLK):
                    ps_sc = scps.tile([P, 256], F32, name="sc_ps", tag="sc_ps")
                    nc.tensor.matmul(ps_sc[:], kt_h[:, tb, :], qT[:, h, :],
                                     start=True, stop=True)
                    nc.scalar.activation(out=expT[:, tb, :], in_=ps_sc[:],
                                         func=AF.Exp, scale=float(SCALE))
                # zero non-causal probabilities (one fused multiply)
                nc.vector.tensor_mul(expT[:], expT[:], mask_sb[:])
                for j in range(2):
                    ps_at = atps.tile([P, D + 1], F32, name="at_ps", tag="at_ps")
                    for tb in range(NBLK):
                        nc.tensor.matmul(
                            ps_at[:], expT[:, tb, j * P:(j + 1) * P],
                            v_all[tb][:, h, :],
                            start=(tb == 0), stop=(tb == NBLK - 1))
                    recip = asmall.tile([P, 1], F32, name="recip", tag="recip")
                    nc.vector.reciprocal(recip[:], ps_at[:, D:D + 1])
                    a_sb = asmall.tile([P, P], BF, name="a_sb", tag="a_sb")
                    nc.vector.tensor_scalar_mul(a_sb[:], ps_at[:, :D], recip[:])
                    tp = trps.tile([P, P], BF, name="tp_at", tag="tp_at")
                    nc.tensor.transpose(tp[:], a_sb[:], ident[:])
                    nc.vector.tensor_copy(attnT[:, h, j * P:(j + 1) * P], tp[:])

        # ---------- P6: attn out-proj + residual ----------
        with tc.tile_pool(name="wpa_sb", bufs=3) as wpa_pool, \
             tc.tile_pool(name="pps", bufs=1, space="PSUM") as pps_pool:
            ps = [[pps_pool.tile([P, 512], F32, name=f"p_ps{j}_{n}",
                                 tag=f"p_ps{j}_{n}") for n in range(4)]
                  for j in range(2)]
            for hc in range(EC):
                wt = wpa_pool.tile([P, 4, 512], BF, name="wpa_t", tag="wpa_t")
                nc.sync.dma_start(wt[:], wpa[hc])
                for j in range(2):
                    for n in range(4):
                        nc.tensor.matmul(ps[j][n][:],
                                         attnT[:, hc, j * P:(j + 1) * P],
                                         wt[:, n, :], start=(hc == 0),
                                         stop=(hc == EC - 1))
            for j in range(2):
                for n in range(4):
                    nc.vector.tensor_add(
                        out=x_sb[j][:, n * 512:(n + 1) * 512],
                        in0=x_sb[j][:, n * 512:(n + 1) * 512], in1=ps[j][n][:])

        # ---------- P7: LN2 -> h2T ----------
        with tc.tile_pool(name="hbf2", bufs=2) as hbf2_pool, \
             tc.tile_pool(name="tps2", bufs=3, space="PSUM") as tps2:
            for j in range(2):
                h_bf = hbf2_pool.tile([P, E], BF, name="h2_bf", tag="h2_bf")
                _layer_norm_to_bf16(nc, small, x_sb[j], h_bf, eps_tile, f"ln2_{j}")
                for e in range(EC):
                    tp = tps2.tile([P, P], BF, name="tp2", tag="tp2")
                    nc.tensor.transpose(tp[:], h_bf[:, e * P:(e + 1) * P], ident[:])
                    nc.vector.tensor_copy(h2T[:, e, j * P:(j + 1) * P], tp[:])

        # ---------- P8: fc + gelu -> gT ----------
        gT = res.tile([P, FC2, 256], BF, name="gT_sb")
        with tc.tile_pool(name="wfc_sb", bufs=3) as wfc_pool, \
             tc.tile_pool(name="fcps", bufs=4, space="PSUM") as fcps:
            for f2 in range(FC2):
                wt = wfc_pool.tile([P, EC, P], BF, name="wfc_t", tag="wfc_t")
                nc.sync.dma_start(wt[:], wfc[f2])
                ps = fcps.tile([P, 256], F32, name="fc_ps", tag="fc_ps")
                for e in range(EC):
                    nc.tensor.matmul(ps[:], wt[:, e, :], h2T[:, e, :],
                                     start=(e == 0), stop=(e == EC - 1))
                nc.scalar.activation(out=gT[:, f2, :], in_=ps[:],
                                     func=AF.Gelu_apprx_tanh)

        # ---------- P9: ffn out-proj + residual, P10: store ----------
        with tc.tile_pool(name="wpf_sb", bufs=4) as wpf_pool, \
             tc.tile_pool(name="pfps", bufs=1, space="PSUM") as pfps:
            ps = [[pfps.tile([P, 512], F32, name=f"pf_ps{j}_{n}",
                             tag=f"pf_ps{j}_{n}") for n in range(4)]
                  for j in range(2)]
            for f2 in range(FC2):
                wt = wpf_pool.tile([P, 4, 512], BF, name="wpf_t", tag="wpf_t")
                nc.sync.dma_start(wt[:], wpf[f2])
                for j in range(2):
                    for n in range(4):
                        nc.tensor.matmul(ps[j][n][:],
                                         gT[:, f2, j * P:(j + 1) * P],
                                         wt[:, n, :], start=(f2 == 0),
                                         stop=(f2 == FC2 - 1))
            for j in range(2):
                for n in range(4):
                    nc.vector.tensor_add(
                        out=x_sb[j][:, n * 512:(n + 1) * 512],
                        in0=x_sb[j][:, n * 512:(n + 1) * 512], in1=ps[j][n][:])
            for j in range(2):
                nc.sync.dma_start(out_own[j], x_sb[j][:])


# ------------------------------------------------------------------
# host side
# ------------------------------------------------------------------
_BF = ml_dtypes.bfloat16


def _prep_shared(ln1_w, ln2_w, w_attn, w_proj_attn, w_fc, w_proj_ffn):
    w_attn = (w_attn * ln1_w[None, :]).astype(np.float32)
    w_fc = (w_fc * ln2_w[None, :]).astype(np.float32)
    wqk = np.ascontiguousarray(
        w_attn[:2 * E].reshape(32, P, EC, P).transpose(0, 3, 2, 1)).astype(_BF)
    # wv[half, e, ei, dci, f]
    wv = np.ascontiguousarray(
        w_attn[2 * E:].reshape(2, 2, 512, EC, P).transpose(0, 3, 4, 1, 2)
    ).astype(_BF)
    # wpa[hc, i, n, f]
    wpa = np.ascontiguousarray(
        w_proj_attn.reshape(4, 512, EC, P).transpose(2, 3, 0, 1)).astype(_BF)
    wfc_t = np.ascontiguousarray(
        w_fc.reshape(FC2, P, EC, P).transpose(0, 3, 2, 1)).astype(_BF)
    # wpf[f2, fi, n, f]
    wpf = np.ascontiguousarray(
        w_proj_ffn.reshape(4, 512, FC2, P).transpose(2, 3, 0, 1)).astype(_BF)
    ident = np.eye(P, dtype=np.float32).astype(_BF)
    return wqk, wv, wpa, wfc_t, wpf, ident


def _core_masks(c):
    """mask[t, tb, s2] with s2 = j*128+s: key tb*128+t <= query (2c+j)*128+s."""
    t = np.arange(P)
    s = np.arange(P)
    masks = np.zeros((P, NBLK, 2, P), np.float32)
    for j in range(2):
        bq = 2 * c + j
        for tb in range(NBLK):
            if tb < bq:
                masks[:, tb, j, :] = 1.0
            elif tb == bq:
                masks[:, tb, j, :] = (t[:, None] <= s[None, :])
    return np.ascontiguousarray(masks.reshape(P, NBLK, 256)).astype(_BF)


_CACHE = {}


def _get_program():
    if "nc" not in _CACHE:
        _CACHE["nc"] = build_program()
    return _CACHE["nc"]


def make_in_maps(x, ln1_w, ln2_w, w_attn, w_proj_attn, w_fc, w_proj_ffn):
    wqk, wv, wpa, wfc_t, wpf, ident = _prep_shared(
        np.asarray(ln1_w, np.float32), np.asarray(ln2_w, np.float32),
        np.asarray(w_attn, np.float32), np.asarray(w_proj_attn, np.float32),
        np.asarray(w_fc, np.float32), np.asarray(w_proj_ffn, np.float32))
    xb = np.ascontiguousarray(np.asarray(x, np.float32).reshape(NBLK, P, E))
    in_maps = []
    for c in range(NCORES):
        in_maps.append({
            "x_own": np.ascontiguousarray(xb[2 * c:2 * c + 2]),
            "wqk": wqk, "wv": wv, "wpa": wpa, "wfc": wfc_t, "wpf": wpf,
            "ident": ident, "mask": _core_masks(c),
        })
    return in_maps


def kernel(x, ln1_w, ln2_w, w_attn, w_proj_attn, w_fc, w_proj_ffn):
    nc = _get_program()
    in_maps = make_in_maps(x, ln1_w, ln2_w, w_attn, w_proj_attn, w_fc,
                           w_proj_ffn)
    res = run_bass_kernel_spmd(nc, in_maps, core_ids=list(range(NCORES)))
    out = np.empty((S, E), np.float32)
    for c in range(NCORES):
        blk = res.results[c]["out_own"]
        out[2 * c * P:(2 * c + 1) * P] = blk[0]
        out[(2 * c + 1) * P:(2 * c + 2) * P] = blk[1]
    return out


if __name__ == "__main__":
    rng = np.random.default_rng(0)
    ins = {
        "x": rng.standard_normal((S, E), dtype=np.float32),
        "ln1_w": np.ones(E, np.float32),
        "ln2_w": np.ones(E, np.float32),
        "w_attn": (rng.standard_normal((3 * E, E), dtype=np.float32) * 0.02),
        "w_proj_attn": (rng.standard_normal((E, E), dtype=np.float32) * 0.02),
        "w_fc": (rng.standard_normal((FH, E), dtype=np.float32) * 0.02),
        "w_proj_ffn": (rng.standard_normal((E, FH), dtype=np.float32) * 0.02),
    }
    out = kernel(**ins)
    print("ran:", out.shape, out.dtype, np.abs(out).max())


# revision 6
# speedup vs baseline: 1.1913x; 1.0078x over previous
"""Trainium2 Bass kernel for a GPT-style decoder block (S=2048, E=2048, H=16, D=128).

Sharding: sequence-parallel across 8 NeuronCores. Core c owns row-blocks
(2c, 2c+1) of 128 rows each. Everything (LN1, QKV, attention queries,
attn-proj, LN2, FFN) is computed row-parallel on the owned 256 rows; the
only collective is a single AllGather of the (bf16) K^T and V projections
so every core can attend over the full key space. Causality is enforced
with per-core mask inputs so the program is SPMD-uniform across cores.

All matmuls run in bf16 with fp32 PSUM accumulation; the residual stream
stays fp32 in SBUF end-to-end. LayerNorm scale vectors are folded into the
following weight matrices on the host (exact for ln_w == 1).
"""

import numpy as np
import ml_dtypes

import concourse.bass as bass
import concourse.mybir as mybir
import concourse.tile as tile
from concourse import bacc
from concourse.bass_utils import run_bass_kernel_spmd

P = 128
S, E, H, D = 2048, 2048, 16, 128
FH = 4 * E          # 8192 ffn hidden
NCORES = 8
NBLK = 16           # S / P row blocks
EC = E // P         # 16 contraction chunks over E
FC2 = FH // P       # 64 f-chunks for the fc matmul
BF = mybir.dt.bfloat16
F32 = mybir.dt.float32
EPS = 1e-5
SCALE = 1.0 / np.sqrt(D)
AF = mybir.ActivationFunctionType
ALU = mybir.AluOpType

KT_ELEMS = H * D * 256          # per-rank K^T contribution elements
V_ELEMS = 256 * E               # per-rank V contribution elements
CONTRIB = KT_ELEMS + V_ELEMS


def _layer_norm_to_bf16(nc, small, x_sb, h_out, eps_tile, tag):
    """h_out[P, E](bf16) = (x - mean(x)) * rsqrt(var(x) + eps), rowwise over E."""
    stats = small.tile([P, 4, 6], F32, name=f"stats_{tag}", tag="stats")
    for g in range(4):
        nc.vector.bn_stats(out=stats[:, g, :], in_=x_sb[:, g * 512:(g + 1) * 512])
    mv = small.tile([P, 2], F32, name=f"mv_{tag}", tag="mv")
    nc.vector.bn_aggr(out=mv[:], in_=stats[:])
    std = small.tile([P, 1], F32, name=f"std_{tag}", tag="std")
    nc.scalar.activation(out=std[:], in_=mv[:, 1:2], func=AF.Sqrt,
                         bias=eps_tile[:], scale=1.0)
    rstd = small.tile([P, 1], F32, name=f"rstd_{tag}", tag="rstd")
    nc.vector.reciprocal(out=rstd[:], in_=std[:])
    nc.vector.tensor_scalar(
        out=h_out[:], in0=x_sb[:], scalar1=mv[:, 0:1], scalar2=rstd[:],
        op0=ALU.subtract, op1=ALU.mult)


def build_program():
    nc = bacc.Bacc()

    # ---- external I/O (per-core views; host pre-tiles the weights) ----
    x_own = nc.dram_tensor("x_own", [2, P, E], F32, kind="ExternalInput")
    # wqk[fc][ei, eo, f] = w_attn[fc*128+f, eo*128+ei] (q rows then k rows)
    wqk = nc.dram_tensor("wqk", [32, P, EC, P], BF, kind="ExternalInput")
    # wv[half][e][ei, dci, f] = w_attn[4096 + (2*half+dci)*512 + f, e*128+ei]
    wv = nc.dram_tensor("wv", [2, EC, P, 2, 512], BF, kind="ExternalInput")
    # wpa[hc][i, n, f] = w_proj_attn[n*512+f, hc*128+i]
    wpa = nc.dram_tensor("wpa", [EC, P, 4, 512], BF, kind="ExternalInput")
    # wfc[f2][ei, eo, f] = w_fc[f2*128+f, eo*128+ei]
    wfc = nc.dram_tensor("wfc", [FC2, P, EC, P], BF, kind="ExternalInput")
    # wpf[f2][fi, n, f] = w_proj_ffn[n*512+f, f2*128+fi]
    wpf = nc.dram_tensor("wpf", [FC2, P, 4, 512], BF, kind="ExternalInput")
    ident_in = nc.dram_tensor("ident", [P, P], BF, kind="ExternalInput")
    # mask[t, tb, s2]: s2 = j*128+s; 1 where key tb*128+t <= query (2c+j)*128+s
    mask_in = nc.dram_tensor("mask", [P, NBLK, 256], BF, kind="ExternalInput")
    out_own = nc.dram_tensor("out_own", [2, P, E], F32, kind="ExternalOutput")

    with tile.TileContext(nc) as tc:
        _body(nc, tc, x_own, wqk, wv, wpa, wfc, wpf, ident_in, mask_in, out_own)
    nc.finalize()
    return nc


def _body(nc, tc, x_own, wqk, wv, wpa, wfc, wpf, ident_in, mask_in, out_own):
    with tc.tile_pool(name="resident", bufs=1) as res, \
         tc.tile_pool(name="small", bufs=4) as small, \
         tc.tile_pool(name="dram", bufs=1, space="DRAM") as dram:

        # ---------- constants / residents ----------
        eps_tile = small.tile([P, 1], F32, name="eps_tile", tag="eps")
        nc.vector.memset(eps_tile[:], EPS)
        ident = res.tile([P, P], BF, name="ident_sb")
        nc.sync.dma_start(ident[:], ident_in[:])
        mask_sb = res.tile([P, NBLK, 256], BF, name="mask_sb")
        nc.sync.dma_start(mask_sb[:], mask_in[:])

        x_sb = []
        for j in range(2):
            x = res.tile([P, E], F32, name=f"x_sb{j}")
            nc.sync.dma_start(x[:], x_own[j])
            x_sb.append(x)

        hT = res.tile([P, EC, 256], BF, name="hT_sb")
        qT = res.tile([P, H, 256], BF, name="qT_sb")
        attnT = res.tile([P, H, 256], BF, name="attnT_sb")
        h2T = res.tile([P, EC, 256], BF, name="h2T_sb")

        cc_in = dram.tile([CONTRIB], BF, name="cc_in")
        cc_out = dram.tile([NCORES, CONTRIB], BF, name="cc_out",
                           addr_space="Shared")
        cc_in_kt = cc_in[:KT_ELEMS].rearrange("(h d s) -> h d s", h=H, d=D)
        cc_in_v = cc_in[KT_ELEMS:].rearrange("(s e) -> s e", s=256)

        # ---------- P1+P2: LN1 -> h (bf16) -> hT ----------
        with tc.tile_pool(name="hbf", bufs=2) as hbf_pool, \
             tc.tile_pool(name="tps1", bufs=3, space="PSUM") as tps1:
            for j in range(2):
                h_bf = hbf_pool.tile([P, E], BF, name="h_bf", tag="h_bf")
                _layer_norm_to_bf16(nc, small, x_sb[j], h_bf, eps_tile, f"ln1_{j}")
                for e in range(EC):
                    tp = tps1.tile([P, P], BF, name="tp1", tag="tp1")
                    nc.tensor.transpose(tp[:], h_bf[:, e * P:(e + 1) * P], ident[:])
                    nc.vector.tensor_copy(hT[:, e, j * P:(j + 1) * P], tp[:])

        # ---------- P3: k, v (to AG input), then q ----------
        with tc.tile_pool(name="wqk_sb", bufs=3) as wqk_pool, \
             tc.tile_pool(name="wv_sb", bufs=4) as wv_pool, \
             tc.tile_pool(name="kvtmp", bufs=3) as kv_pool, \
             tc.tile_pool(name="qkps", bufs=2, space="PSUM") as qkps, \
             tc.tile_pool(name="vps", bufs=1, space="PSUM") as vps:

            def qk_chunk(fc):
                wt = wqk_pool.tile([P, EC, P], BF, name="wqk_t", tag="wqk_t")
                nc.scalar.dma_start(wt[:], wqk[fc])
                ps = qkps.tile([P, 256], F32, name="qk_ps", tag="qk_ps")
                for e in range(EC):
                    nc.tensor.matmul(ps[:], wt[:, e, :], hT[:, e, :],
                                     start=(e == 0), stop=(e == EC - 1))
                return ps

            # k chunks first so the AllGather can start as early as possible
            for h in range(H):
                ps = qk_chunk(16 + h)
                kt_sb = kv_pool.tile([P, 256], BF, name="kt_sb", tag="kt_sb")
                nc.vector.tensor_copy(kt_sb[:], ps[:])
                nc.sync.dma_start(cc_in_kt[h], kt_sb[:])

            # v: out[s, d] with lhsT = hT chunk, rhs = wv tile
            for half in range(2):
                ps_v = [[vps.tile([P, 512], F32, name=f"v_ps{j}_{dci}",
                                  tag=f"v_ps{j}_{dci}") for dci in range(2)]
                        for j in range(2)]
                for e in range(EC):
                    wvt = wv_pool.tile([P, 2, 512], BF, name="wv_t", tag="wv_t")
                    nc.sync.dma_start(wvt[:], wv[half, e])
                    for j in range(2):
                        for dci in range(2):
                            nc.tensor.matmul(
                                ps_v[j][dci][:], hT[:, e, j * P:(j + 1) * P],
                                wvt[:, dci, :],
                                start=(e == 0), stop=(e == EC - 1))
                for j in range(2):
                    for dci in range(2):
                        dc = 2 * half + dci
                        v_sb = kv_pool.tile([P, 512], BF, name="v_sb", tag="v_sb")
                        nc.vector.tensor_copy(v_sb[:], ps_v[j][dci][:])
                        nc.sync.dma_start(
                            cc_in_v[j * P:(j + 1) * P, dc * 512:(dc + 1) * 512],
                            v_sb[:])

            nc.gpsimd.collective_compute(
                "AllGather", ALU.bypass,
                replica_groups=[list(range(NCORES))],
                ins=[cc_in.opt()], outs=[cc_out.opt()])

            # q chunks overlap the AllGather
            for h in range(H):
                ps = qk_chunk(h)
                nc.vector.tensor_copy(qT[:, h, :], ps[:])

        ccout_kt = cc_out[:, :KT_ELEMS].rearrange(
            "r (h d b s) -> r h d b s", h=H, d=D, b=2)
        ccout_v = cc_out[:, KT_ELEMS:].rearrange(
            "r (b t e) -> r b t e", b=2, t=P)

        # ---------- P5: attention ----------
        with tc.tile_pool(name="vall", bufs=1) as vall_pool, \
             tc.tile_pool(name="kth", bufs=3) as kt_pool, \
             tc.tile_pool(name="expt", bufs=2) as exp_pool, \
             tc.tile_pool(name="attn_small", bufs=4) as asmall, \
             tc.tile_pool(name="scps", bufs=3, space="PSUM") as scps, \
             tc.tile_pool(name="atps", bufs=2, space="PSUM") as atps, \
             tc.tile_pool(name="trps", bufs=2, space="PSUM") as trps:

            # V for all heads, resident: V_all[tb][t, h, 0:128]=V, [:, :, 128]=1
            v_all = []
            for tb in range(NBLK):
                r, b = tb // 2, tb % 2
                vt = vall_pool.tile([P, H, D + 1], BF, name=f"v_all_{tb}")
                nc.sync.dma_start(
                    vt[:, :, :D], ccout_v[r, b].rearrange("t (h d) -> t h d", h=H))
                nc.vector.memset(vt[:, :, D:D + 1], 1.0)
                v_all.append(vt)

            for h in range(H):
                kt_h = kt_pool.tile([P, NBLK, P], BF, name="kt_h", tag="kt_h")
                nc.sync.dma_start(
                    kt_h[:], ccout_kt[:, h].rearrange("r d b s -> d r b s"))
                # scores + exp for both query blocks at once (free dim 256)
                expT = exp_pool.tile([P, NBLK, 256], BF, name="expT", tag="expT")
                for tb in range(NBLK):
                    ps_sc = scps.tile([P, 256], F32, name="sc_ps", tag="sc_ps")
                    nc.tensor.matmul(ps_sc[:], kt_h[:, tb, :], qT[:, h, :],
                                     start=True, stop=True)
                    nc.scalar.activation(out=expT[:, tb, :], in_=ps_sc[:],
                                         func=AF.Exp, scale=float(SCALE))
                # zero non-causal probabilities (one fused multiply)
                nc.vector.tensor_mul(expT[:], expT[:], mask_sb[:])
                for j in range(2):
                    ps_at = atps.tile([P, D + 1], F32, name="at_ps", tag="at_ps")
                    for tb in range(NBLK):
                        nc.tensor.matmul(
                            ps_at[:], expT[:, tb, j * P:(j + 1) * P],
                            v_all[tb][:, h, :],
                            start=(tb == 0), stop=(tb == NBLK - 1))
                    recip = asmall.tile([P, 1], F32, name="recip", tag="recip")
                    nc.vector.reciprocal(recip[:], ps_at[:, D:D + 1])
                    a_sb = asmall.tile([P, P], BF, name="a_sb", tag="a_sb")
                    nc.vector.tensor_scalar_mul(a_sb[:], ps_at[:, :D], recip[:])
                    tp = trps.tile([P, P], BF, name="tp_at", tag="tp_at")
                    nc.tensor.transpose(tp[:], a_sb[:], ident[:])
                    nc.vector.tensor_copy(attnT[:, h, j * P:(j + 1) * P], tp[:])

        # ---------- P6: attn out-proj + residual ----------
        with tc.tile_pool(name="wpa_sb", bufs=3) as wpa_pool, \
             tc.tile_pool(name="pps", bufs=1, space="PSUM") as pps_pool:
            ps = [[pps_pool.tile([P, 512], F32, name=f"p_ps{j}_{n}",
                                 tag=f"p_ps{j}_{n}") for n in range(4)]
                  for j in range(2)]
            for hc in range(EC):
                wt = wpa_pool.tile([P, 4, 512], BF, name="wpa_t", tag="wpa_t")
                nc.sync.dma_start(wt[:], wpa[hc])
                for j in range(2):
                    for n in range(4):
                        nc.tensor.matmul(ps[j][n][:],
                                         attnT[:, hc, j * P:(j + 1) * P],
                                         wt[:, n, :], start=(hc == 0),
                                         stop=(hc == EC - 1))
            for j in range(2):
                for n in range(4):
                    nc.vector.tensor_add(
                        out=x_sb[j][:, n * 512:(n + 1) * 512],
                        in0=x_sb[j][:, n * 512:(n + 1) * 512], in1=ps[j][n][:])

        # ---------- P7: LN2 -> h2T ----------
        with tc.tile_pool(name="hbf2", bufs=2) as hbf2_pool, \
             tc.tile_pool(name="tps2", bufs=3, space="PSUM") as tps2:
            for j in range(2):
                h_bf = hbf2_pool.tile([P, E], BF, name="h2_bf", tag="h2_bf")
                _layer_norm_to_bf16(nc, small, x_sb[j], h_bf, eps_tile, f"ln2_{j}")
                for e in range(EC):
                    tp = tps2.tile([P, P], BF, name="tp2", tag="tp2")
                    nc.tensor.transpose(tp[:], h_bf[:, e * P:(e + 1) * P], ident[:])
                    nc.vector.tensor_copy(h2T[:, e, j * P:(j + 1) * P], tp[:])

        # ---------- P8: fc + gelu -> gT ----------
        gT = res.tile([P, FC2, 256], BF, name="gT_sb")
        with tc.tile_pool(name="wfc_sb", bufs=3) as wfc_pool, \
             tc.tile_pool(name="fcps", bufs=4, space="PSUM") as fcps:
            for f2 in range(FC2):
                wt = wfc_pool.tile([P, EC, P], BF, name="wfc_t", tag="wfc_t")
                nc.scalar.dma_start(wt[:], wfc[f2])
                ps = fcps.tile([P, 256], F32, name="fc_ps", tag="fc_ps")
                for e in range(EC):
                    nc.tensor.matmul(ps[:], wt[:, e, :], h2T[:, e, :],
                                     start=(e == 0), stop=(e == EC - 1))
                nc.scalar.activation(out=gT[:, f2, :], in_=ps[:],
                                     func=AF.Gelu_apprx_tanh)

        # ---------- P9: ffn out-proj + residual, P10: store ----------
        with tc.tile_pool(name="wpf_sb", bufs=4) as wpf_pool, \
             tc.tile_pool(name="pfps", bufs=1, space="PSUM") as pfps:
            ps = [[pfps.tile([P, 512], F32, name=f"pf_ps{j}_{n}",
                             tag=f"pf_ps{j}_{n}") for n in range(4)]
                  for j in range(2)]
            for f2 in range(FC2):
                wt = wpf_pool.tile([P, 4, 512], BF, name="wpf_t", tag="wpf_t")
                nc.sync.dma_start(wt[:], wpf[f2])
                for j in range(2):
                    for n in range(4):
                        nc.tensor.matmul(ps[j][n][:],
                                         gT[:, f2, j * P:(j + 1) * P],
                                         wt[:, n, :], start=(f2 == 0),
                                         stop=(f2 == FC2 - 1))
            for j in range(2):
                for n in range(4):
                    nc.vector.tensor_add(
                        out=x_sb[j][:, n * 512:(n + 1) * 512],
                        in0=x_sb[j][:, n * 512:(n + 1) * 512], in1=ps[j][n][:])
            for j in range(2):
                nc.sync.dma_start(out_own[j], x_sb[j][:])


# ------------------------------------------------------------------
# host side
# ------------------------------------------------------------------
_BF = ml_dtypes.bfloat16


def _prep_shared(ln1_w, ln2_w, w_attn, w_proj_attn, w_fc, w_proj_ffn):
    w_attn = (w_attn * ln1_w[None, :]).astype(np.float32)
    w_fc = (w_fc * ln2_w[None, :]).astype(np.float32)
    wqk = np.ascontiguousarray(
        w_attn[:2 * E].reshape(32, P, EC, P).transpose(0, 3, 2, 1)).astype(_BF)
    # wv[half, e, ei, dci, f]
    wv = np.ascontiguousarray(
        w_attn[2 * E:].reshape(2, 2, 512, EC, P).transpose(0, 3, 4, 1, 2)
    ).astype(_BF)
    # wpa[hc, i, n, f]
    wpa = np.ascontiguousarray(
        w_proj_attn.reshape(4, 512, EC, P).transpose(2, 3, 0, 1)).astype(_BF)
    wfc_t = np.ascontiguousarray(
        w_fc.reshape(FC2, P, EC, P).transpose(0, 3, 2, 1)).astype(_BF)
    # wpf[f2, fi, n, f]
    wpf = np.ascontiguousarray(
        w_proj_ffn.reshape(4, 512, FC2, P).transpose(2, 3, 0, 1)).astype(_BF)
    ident = np.eye(P, dtype=np.float32).astype(_BF)
    return wqk, wv, wpa, wfc_t, wpf, ident


def _core_masks(c):
    """mask[t, tb, s2] with s2 = j*128+s: key tb*128+t <= query (2c+j)*128+s."""
    t = np.arange(P)
    s = np.arange(P)
    masks = np.zeros((P, NBLK, 2, P), np.float32)
    for j in range(2):
        bq = 2 * c + j
        for tb in range(NBLK):
            if tb < bq:
                masks[:, tb, j, :] = 1.0
            elif tb == bq:
                masks[:, tb, j, :] = (t[:, None] <= s[None, :])
    return np.ascontiguousarray(masks.reshape(P, NBLK, 256)).astype(_BF)


_CACHE = {}


def _get_program():
    if "nc" not in _CACHE:
        _CACHE["nc"] = build_program()
    return _CACHE["nc"]


def make_in_maps(x, ln1_w, ln2_w, w_attn, w_proj_attn, w_fc, w_proj_ffn):
    wqk, wv, wpa, wfc_t, wpf, ident = _prep_shared(
        np.asarray(ln1_w, np.float32), np.asarray(ln2_w, np.float32),
        np.asarray(w_attn, np.float32), np.asarray(w_proj_attn, np.float32),
        np.asarray(w_fc, np.float32), np.asarray(w_proj_ffn, np.float32))
    xb = np.ascontiguousarray(np.asarray(x, np.float32).reshape(NBLK, P, E))
    in_maps = []
    for c in range(NCORES):
        in_maps.append({
            "x_own": np.ascontiguousarray(xb[2 * c:2 * c + 2]),
            "wqk": wqk, "wv": wv, "wpa": wpa, "wfc": wfc_t, "wpf": wpf,
            "ident": ident, "mask": _core_masks(c),
        })
    return in_maps


def kernel(x, ln1_w, ln2_w, w_attn, w_proj_attn, w_fc, w_proj_ffn):
    nc = _get_program()
    in_maps = make_in_maps(x, ln1_w, ln2_w, w_attn, w_proj_attn, w_fc,
                           w_proj_ffn)
    res = run_bass_kernel_spmd(nc, in_maps, core_ids=list(range(NCORES)))
    out = np.empty((S, E), np.float32)
    for c in range(NCORES):
        blk = res.results[c]["out_own"]
        out[2 * c * P:(2 * c + 1) * P] = blk[0]
        out[(2 * c + 1) * P:(2 * c + 2) * P] = blk[1]
    return out


if __name__ == "__main__":
    rng = np.random.default_rng(0)
    ins = {
        "x": rng.standard_normal((S, E), dtype=np.float32),
        "ln1_w": np.ones(E, np.float32),
        "ln2_w": np.ones(E, np.float32),
        "w_attn": (rng.standard_normal((3 * E, E), dtype=np.float32) * 0.02),
        "w_proj_attn": (rng.standard_normal((E, E), dtype=np.float32) * 0.02),
        "w_fc": (rng.standard_normal((FH, E), dtype=np.float32) * 0.02),
        "w_proj_ffn": (rng.standard_normal((E, FH), dtype=np.float32) * 0.02),
    }
    out = kernel(**ins)
    print("ran:", out.shape, out.dtype, np.abs(out).max())
